# revision 9
# baseline (speedup 1.0000x reference)
"""NeuralFactorizationMachine Trainium2 kernel v4 (8 NeuronCores, SPMD).

Reference computation (B=1024, N=16384, D=512, O=4096):
    sum_emb = sae @ emb                      (B, D)
    sum_sq  = (sae*sae) @ (emb*emb)          (B, D)
    inter   = 0.5*(sum_emb^2 - sum_sq)       (B, D)
    h       = relu(inter @ mlp1_w.T + b1)    (B, D)
    out     = h @ mlp2_w.T + b2 + sae @ linear_w.T + lb   (B, O)

v4 = v3 (fp8 DoubleRow linear, mean-shift folded) + N-parallel FM:
  - The FM GEMMs are sharded over the CONTRACTION dim (16 of 128
    k-tiles per core) instead of batch, so each core reads only 1/8 of
    emb (2 MiB vs 16) and 1/8 of sae-bf16 (4 MiB, all batch columns).
    Partial (B, 2D) sums are drained to bf16 and combined with a
    ReduceScatter(add) -- each core receives exactly its 128-row batch
    shard of the true sums (~17us exposed, hidden behind the linear
    stream).  Measured end-to-end max-rel 1.28e-2 (gate 2e-2).
  - DMA drops from ~46 MiB/core (v3) to ~35 MiB/core; per-core HBM
    bandwidth saturates at ~285 GB/s (measured), so this is the
    dominant term.
  - PSUM: the linear stream needs its 8 banks for the whole k-range,
    but mlp1 needs one transiently.  m-tile 0 of the linear GEMM runs
    from a private contiguous side-stream (saeTd is host-swizzled
    (m, kt, b)-major so m0's columns are one 2 MiB read) AFTER mlp1
    releases its bank; the other 7 m-tiles stream normally.  interT
    comes from SBUF->SBUF DMA transposes (no PE/PSUM needed).
  - Engine/queue plan: sync = sfull + pin partial stores + RS + rsin +
    interT transposes + h_mine + sam0 + out; scalar(ACT) = ew + consts
    + sa stream + htall transposes + final drains; gpsimd = lw stream
    (never behind a collective) + AllGather trigger.  DVE does all FM
    squares, partial drains, inter, and relu so the ACT queue is pure
    DMA until the tail.
"""

import numpy as np
import ml_dtypes

import concourse.bass as bass
import concourse.mybir as mybir
import concourse.tile as tile
from concourse import bacc
from concourse.bass_utils import run_bass_kernel_spmd

B, N, D, O = 1024, 16384, 512, 4096
C = 8                # cores
BS = B // C          # 128 batch rows per core (batch shard after RS)
OS = O // C          # 512 output cols per core
BF16 = mybir.dt.bfloat16
FP8 = mybir.dt.float8e4
F32 = mybir.dt.float32

KT = N // 128        # 128 n-ktiles total
KTL = KT // C        # 16 local ktiles per core (FM contraction shard)
DT = D // 128        # 4 d-tiles
MT = B // 128        # 8 m-tiles (batch) for p2
NB = 16              # n-blocks for the linear GEMM stream
NTB = KT // NB       # 8 n-tiles per block
SFP = 8              # sfull load split
D2 = 2 * D

LIN_SCALE = 4096.0   # 8 (sae shift scale) * 512 (w scale)


def _build(repeat=1, phases=("fm", "ag", "p2", "tail")):
    nc = bacc.Bacc(
        "TRN2",
        target_bir_lowering=False,
        debug=False,
        enable_asserts=False,
        num_devices=C,
    )

    # saeTd is (m, kt, b)-major: saeTd[p, ((m*KT)+kt)*128 + b] =
    #   sae8[kt*128+p, m*128+b]
    saeTd = nc.dram_tensor("saeTd", [128, MT * KT * 128], FP8,
                           kind="ExternalInput").ap()
    # per-core FM shards (host pre-sliced): 16 ktiles, kt-major
    saebf = nc.dram_tensor("saebf", [128, KTL * B], BF16,
                           kind="ExternalInput").ap()
    embd = nc.dram_tensor("embd", [128, KTL * D], BF16,
                          kind="ExternalInput").ap()
    linwd = nc.dram_tensor("linwd", [128, KT * OS], FP8,
                           kind="ExternalInput").ap()
    mlp1wT = nc.dram_tensor("mlp1wT", [D, D], BF16, kind="ExternalInput").ap()
    mlp1brow = nc.dram_tensor("mlp1brow", [1, D], BF16,
                              kind="ExternalInput").ap()
    mlp2wT = nc.dram_tensor("mlp2wT", [D, OS], BF16, kind="ExternalInput").ap()
    biasrow = nc.dram_tensor("biasrow", [1, OS], BF16,
                             kind="ExternalInput").ap()
    out = nc.dram_tensor("out", [B, OS], BF16, kind="ExternalOutput").ap()

    saeT4 = saeTd.rearrange("p (m k b) -> p m k b", m=MT, k=KT)

    with tile.TileContext(nc) as tc:
      for rep in range(repeat):
        with (
            tc.tile_pool(name=f"dram{rep}", bufs=1, space="DRAM") as dram,
            tc.tile_pool(name=f"const{rep}", bufs=1) as cst,
            tc.tile_pool(name=f"p2sa{rep}", bufs=3) as p2sa,
            tc.tile_pool(name=f"p2lw{rep}", bufs=7) as p2lw,
            tc.tile_pool(name=f"p2st{rep}", bufs=4) as p2st,
        ):
            # ---- first 7 lw blocks emitted BEFORE the FM section so they
            # sit ahead of the ReduceScatter in the gpsimd instruction
            # stream (the RS blocks gpsimd ~63..80us waiting on the FM
            # partials; blocks 7.. are emitted after it and issue at ~80,
            # still ahead of when the PE needs them)
            LW_PRE = 7
            lw_tiles = []

            def emit_lw(nb):
                lw = p2lw.tile([128, NTB, OS], FP8, tag="lw", name="lw")
                nc.gpsimd.dma_start(
                    lw[:], linwd[:, nb * NTB * OS:(nb + 1) * NTB * OS])
                lw_tiles.append(lw)

            if "p2" in phases:
                for nb in range(LW_PRE):
                    emit_lw(nb)

            # const tiles (scalar queue, behind the first ew chunks)
            w1 = cst.tile([128, DT * D], BF16, tag="w1", name="w1")
            b1r = cst.tile([1, D], BF16, tag="b1r", name="b1r")
            w2 = cst.tile([128, DT * OS], BF16, tag="w2", name="w2")
            br = cst.tile([1, OS], BF16, tag="br", name="br")
            ones = cst.tile([1, 128], BF16, tag="ones", name="ones")
            nc.vector.memset(ones[:], 1.0)

            # h gathered from all cores: [B, D] bf16 (AllGather output)
            h_all = dram.tile([C * BS, D], BF16, tag="h_all",
                              name=f"h_all{rep}", addr_space="Shared")
            h_mine = dram.tile([BS, D], BF16, tag="h_mine",
                               name=f"h_mine{rep}")
            htall = cst.tile([128, DT * B], BF16, tag="htall", name="htall")

            # FM partial-sum exchange buffers
            pin = dram.tile([B, D2], BF16, tag="pin", name=f"pin{rep}")
            rsd = dram.tile([BS, D2], BF16, tag="rsd", name=f"rsd{rep}")

            rsin = cst.tile([128, D2], BF16, tag="rsin", name="rsin")
            inter = cst.tile([128, D], BF16, tag="inter", name="inter")
            interT = cst.tile([128, D], BF16, tag="interT", name="interT")
            hbf = cst.tile([128, D], BF16, tag="hbf", name="hbf")

            # ---------------- Phase FM: N-sharded FM GEMMs ----------------
            if "fm" in phases:
              with (
                tc.tile_pool(name=f"fmw{rep}", bufs=1) as fmw,
                tc.tile_pool(name=f"fmps{rep}", bufs=2, space="PSUM") as fmps,
                tc.tile_pool(name=f"fmst{rep}", bufs=3) as fmst,
              ):
                # ew first on scalar so FM matmuls can start early
                ew = fmw.tile([128, KTL, D], BF16, tag="ew", name="ew")
                esq = fmw.tile([128, KTL, D], BF16, tag="esq", name="esq")
                for chv in range(4):
                    ksl = slice(chv * 4, (chv + 1) * 4)
                    nc.scalar.dma_start(ew[:, ksl, :],
                                        embd[:, chv * 4 * D:(chv + 1) * 4 * D])
                    nc.vector.tensor_mul(esq[:, ksl, :], ew[:, ksl, :],
                                         ew[:, ksl, :])

                sfull = fmw.tile([128, KTL, B], BF16, tag="sfull",
                                 name="sfull")
                ssq = fmw.tile([128, KTL, B], BF16, tag="ssq", name="ssq")
                kpp = KTL // SFP
                for s in range(SFP):
                    ksl = slice(s * kpp, (s + 1) * kpp)
                    nc.sync.dma_start(
                        sfull[:, ksl, :],
                        saebf[:, s * kpp * B:(s + 1) * kpp * B])
                    nc.vector.tensor_mul(ssq[:, ksl, :], sfull[:, ksl, :],
                                         sfull[:, ksl, :])

                nc.scalar.dma_start(
                    w1[:], mlp1wT.rearrange("(k p) d -> p k d", p=128))
                nc.scalar.dma_start(b1r[:], mlp1brow[:, :])
                nc.scalar.dma_start(
                    w2[:], mlp2wT.rearrange("(k p) o -> p k o", p=128))
                nc.scalar.dma_start(br[:], biasrow[:, :])

                for m in range(MT):
                    msl = slice(m * 128, (m + 1) * 128)
                    pse = fmps.tile([128, D], F32, tag="pse", name="pse")
                    psq = fmps.tile([128, D], F32, tag="psq", name="psq")
                    for k in range(KTL):
                        nc.tensor.matmul(
                            pse[:], sfull[:, k, msl], ew[:, k, :],
                            start=(k == 0), stop=(k == KTL - 1),
                            skip_group_check=True,
                        )
                        nc.tensor.matmul(
                            psq[:], ssq[:, k, msl], esq[:, k, :],
                            start=(k == 0), stop=(k == KTL - 1),
                            skip_group_check=True,
                        )
                    part = fmst.tile([128, D2], BF16, tag="part", name="part")
                    nc.vector.tensor_copy(part[:, :D], pse[:])
                    nc.vector.tensor_copy(part[:, D:], psq[:])
                    nc.sync.dma_start(pin[m * 128:(m + 1) * 128, :], part[:])

              # sum partials across cores; core c receives rows c*128..
              nc.gpsimd.collective_compute(
                  "ReduceScatter",
                  mybir.AluOpType.add,
                  replica_groups=[list(range(C))],
                  ins=[pin.opt()],
                  outs=[rsd.opt()],
              )
              nc.sync.dma_start(rsin[:], rsd[:, :])

              # inter = sum_emb^2 - sum_sq (0.5 folded into w1), on DVE
              itmp = cst.tile([128, D], F32, tag="itmp", name="itmp")
              nc.vector.tensor_mul(itmp[:], rsin[:, :D], rsin[:, :D])
              nc.vector.tensor_sub(inter[:], itmp[:], rsin[:, D:])
              # interT via SBUF->SBUF DMA transpose (no PE/PSUM)
              for dc in range(DT):
                  nc.sync.dma_start_transpose(
                      interT[:, dc * 128:(dc + 1) * 128],
                      inter[:, dc * 128:(dc + 1) * 128])
            else:
                nc.scalar.dma_start(
                    w1[:], mlp1wT.rearrange("(k p) d -> p k d", p=128))
                nc.scalar.dma_start(b1r[:], mlp1brow[:, :])
                nc.scalar.dma_start(
                    w2[:], mlp2wT.rearrange("(k p) o -> p k o", p=128))
                nc.scalar.dma_start(br[:], biasrow[:, :])
                nc.vector.memset(interT[:], 0.01)

            # ---------------- p2 PSUM pools + mlp1 interleave -------------
            # stack: psm7..psm1 (7 banks), hps (1 bank, transient for mlp1),
            # then psm0 takes hps' bank after mlp1 completes.
            p2ps_cms = [None] * MT
            p2ps_pools = [None] * MT
            for m in range(MT - 1, 0, -1):
                p2ps_cms[m] = tc.tile_pool(
                    name=f"p2ps{rep}_{m}", bufs=1, space="PSUM")
                p2ps_pools[m] = p2ps_cms[m].__enter__()
            psm = [None] * MT
            for m in range(1, MT):
                psm[m] = p2ps_pools[m].tile([128, OS], F32, tag=f"psm{m}",
                                            name=f"psm{m}")

            # mlp1: h = relu(interT.T @ w1 + b1) for OUR batch shard
            hps_cm = tc.tile_pool(name=f"hps{rep}", bufs=1, space="PSUM")
            hps_pool = hps_cm.__enter__()
            hps = hps_pool.tile([128, D], F32, tag="hps", name="hps")
            for kd in range(DT):
                nc.tensor.matmul(
                    hps[:], interT[:, kd * 128:(kd + 1) * 128],
                    w1[:, kd * D:(kd + 1) * D],
                    start=(kd == 0), stop=False, skip_group_check=True,
                )
            nc.tensor.matmul(
                hps[:], ones[:, :], b1r[:, :],
                start=False, stop=True, skip_group_check=True,
            )
            # relu on DVE (ACT queue stays pure-DMA until the tail)
            nc.vector.tensor_scalar_max(hbf[:], hps[:], 0.0)
            nc.sync.dma_start(h_mine[:, :], hbf[:])
            hps_cm.__exit__(None, None, None)

            p2ps_cms[0] = tc.tile_pool(name=f"p2ps{rep}_0", bufs=1,
                                       space="PSUM")
            p2ps_pools[0] = p2ps_cms[0].__enter__()
            psm[0] = p2ps_pools[0].tile([128, OS], F32, tag="psm0",
                                        name="psm0")

            # ---------------- Phase p2: fp8 DoubleRow linear GEMM ---------
            if "p2" in phases:
                # m-tiles 1..7 ride the shared sa stream
                for nb in range(NB):
                    if nb + LW_PRE < NB:
                        emit_lw(nb + LW_PRE)
                    sa = p2sa.tile([128, MT - 1, NTB, 128], FP8, tag="sa",
                                   name="sa")
                    nc.scalar.dma_start(
                        sa[:], saeT4[:, 1:, nb * NTB:(nb + 1) * NTB, :])
                    lw = lw_tiles[nb]
                    for mi in range(MT - 1):
                        for jp in range(NTB // 2):
                            nc.tensor.matmul(
                                psm[mi + 1][:],
                                sa[:, mi, 2 * jp:2 * jp + 2, :],
                                lw[:, 2 * jp:2 * jp + 2, :],
                                start=(nb == 0 and jp == 0),
                                stop=False,
                                perf_mode=mybir.MatmulPerfMode.DoubleRow,
                                skip_group_check=True,
                            )

                # m0 from its private contiguous side-stream (after mlp1
                # released its PSUM bank)
                sam0 = cst.tile([128, KT, 128], FP8, tag="sam0", name="sam0")
                for hh in range(2):
                    ksl = slice(hh * (KT // 2), (hh + 1) * (KT // 2))
                    nc.sync.dma_start(sam0[:, ksl, :], saeT4[:, 0, ksl, :])
                for nb in range(NB):
                    for jp in range(NTB // 2):
                        j0 = nb * NTB + 2 * jp
                        nc.tensor.matmul(
                            psm[0][:],
                            sam0[:, j0:j0 + 2, :],
                            lw_tiles[nb][:, 2 * jp:2 * jp + 2, :],
                            start=(nb == 0 and jp == 0),
                            stop=False,
                            perf_mode=mybir.MatmulPerfMode.DoubleRow,
                            skip_group_check=True,
                        )
            else:
                for m in range(MT):
                    nc.tensor.matmul(
                        psm[m][:], ones[:, :], br[:, :],
                        start=True, stop=False, skip_group_check=True,
                    )

            # AllGather of h (emitted after the lw stream: its wait on h
            # cannot head-block anything that matters on gpsimd now)
            if "ag" in phases and "fm" in phases:
                nc.gpsimd.collective_compute(
                    "AllGather",
                    mybir.AluOpType.bypass,
                    replica_groups=[list(range(C))],
                    ins=[h_mine.opt()],
                    outs=[h_all.opt()],
                )
                for kd in range(DT):
                    nc.scalar.dma_start_transpose(
                        htall[:, kd * B:(kd + 1) * B],
                        h_all[:, kd * 128:(kd + 1) * 128])
            else:
                nc.vector.memset(htall[:], 0.01)

            # ---------------- tail: mlp2 + bias + drain -------------------
            for m in range(MT):
                if "tail" in phases:
                    for kd in range(DT):
                        nc.tensor.matmul(
                            psm[m][:],
                            htall[:, kd * B + m * 128:kd * B + (m + 1) * 128],
                            w2[:, kd * OS:(kd + 1) * OS],
                            start=False, stop=False,
                            skip_group_check=True,
                        )
                nc.tensor.matmul(
                    psm[m][:], ones[:, :], br[:, :],
                    start=False, stop=True, skip_group_check=True,
                )
                ot = p2st.tile([128, OS], BF16, tag="ot", name="ot")
                nc.scalar.activation(
                    ot[:], psm[m][:],
                    mybir.ActivationFunctionType.Copy,
                    scale=1.0 / LIN_SCALE)
                nc.sync.dma_start(out[m * 128:(m + 1) * 128, :], ot[:])
                p2ps_cms[m].__exit__(None, None, None)

    nc.compile()
    return nc


_CACHE = {}


def _get_nc():
    if "nc" not in _CACHE:
        _CACHE["nc"] = _build()
    return _CACHE["nc"]


def _swz(a, inner):
    """[K*128, inner] row-major -> [128, K*inner] partition-major flat."""
    k = a.shape[0] // 128
    return np.ascontiguousarray(
        a.reshape(k, 128, inner).transpose(1, 0, 2).reshape(128, k * inner))


def make_in_maps(sae_features, emb, linear_w, linear_b, mlp1_w, mlp1_b,
                 mlp2_w, mlp2_b):
    bf = ml_dtypes.bfloat16
    e4 = ml_dtypes.float8_e4m3
    f32 = np.float32
    sae = np.asarray(sae_features, dtype=f32)
    emb_f = np.asarray(emb, dtype=f32)
    W = np.asarray(linear_w, f32)

    saeT = np.ascontiguousarray(sae.T)              # (N, B) f32
    saeT8 = ((saeT - 0.5) * 8.0).astype(e4)         # (N, B) fp8, mean-shifted
    # (m, kt, b)-major fp8 layout: [128, MT*KT*128]
    saeTd = np.ascontiguousarray(
        saeT8.reshape(KT, 128, MT, 128).transpose(1, 2, 0, 3)
        .reshape(128, MT * KT * 128))
    saeT_bf = saeT.astype(bf)                        # (N, B) bf16
    embd_full = _swz(emb_f.astype(bf), D)            # [128, KT*D]
    saebf_full = _swz(saeT_bf, B)                    # [128, KT*B]
    mlp1wT = np.ascontiguousarray((0.5 * np.asarray(mlp1_w, f32)).T).astype(bf)
    mlp1brow = np.asarray(mlp1_b, f32).reshape(1, D).astype(bf)
    mlp2wT_f = np.ascontiguousarray(np.asarray(mlp2_w, f32).T) * LIN_SCALE
    linwT_f = np.ascontiguousarray(W.T) * 512.0     # (N, O) f32, fp8-scaled
    # fused bias: linear_b + mlp2_b + the mean-shift correction, PSUM scale
    bias_f = (np.asarray(linear_b, f32) + np.asarray(mlp2_b, f32)
              + 0.5 * W.sum(axis=1)) * LIN_SCALE    # (O,)

    in_maps = []
    for c in range(C):
        osl = slice(c * OS, (c + 1) * OS)
        in_maps.append({
            "saeTd": saeTd,
            "saebf": np.ascontiguousarray(
                saebf_full[:, c * KTL * B:(c + 1) * KTL * B]),
            "embd": np.ascontiguousarray(
                embd_full[:, c * KTL * D:(c + 1) * KTL * D]),
            "linwd": _swz(
                np.ascontiguousarray(linwT_f[:, osl]).astype(e4), OS),
            "mlp1wT": mlp1wT,
            "mlp1brow": mlp1brow,
            "mlp2wT": np.ascontiguousarray(mlp2wT_f[:, osl]).astype(bf),
            "biasrow": bias_f[osl].reshape(1, OS).astype(bf),
        })
    return in_maps


def kernel(sae_features, emb, linear_w, linear_b, mlp1_w, mlp1_b, mlp2_w,
           mlp2_b):
    nc = _get_nc()
    in_maps = make_in_maps(
        sae_features, emb, linear_w, linear_b, mlp1_w, mlp1_b, mlp2_w, mlp2_b
    )
    res = run_bass_kernel_spmd(nc, in_maps, list(range(C)))
    full = np.empty((B, O), dtype=np.float32)
    for c in range(C):
        full[:, c * OS:(c + 1) * OS] = res.results[c]["out"].astype(np.float32)
    return full


# revision 14
# speedup vs baseline: 1.0822x; 1.0822x over previous
"""NeuralFactorizationMachine Trainium2 kernel v4 (8 NeuronCores, SPMD).

Reference computation (B=1024, N=16384, D=512, O=4096):
    sum_emb = sae @ emb                      (B, D)
    sum_sq  = (sae*sae) @ (emb*emb)          (B, D)
    inter   = 0.5*(sum_emb^2 - sum_sq)       (B, D)
    h       = relu(inter @ mlp1_w.T + b1)    (B, D)
    out     = h @ mlp2_w.T + b2 + sae @ linear_w.T + lb   (B, O)

v4 = v3 (fp8 DoubleRow linear, mean-shift folded) + N-parallel FM:
  - The FM GEMMs are sharded over the CONTRACTION dim (16 of 128
    k-tiles per core) instead of batch, so each core reads only 1/8 of
    emb (2 MiB vs 16) and 1/8 of sae-bf16 (4 MiB, all batch columns).
    Partial (B, 2D) sums are drained to bf16 and combined with a
    ReduceScatter(add) -- each core receives exactly its 128-row batch
    shard of the true sums (~17us exposed, hidden behind the linear
    stream).  Measured end-to-end max-rel 1.28e-2 (gate 2e-2).
  - DMA drops from ~46 MiB/core (v3) to ~35 MiB/core; per-core HBM
    bandwidth saturates at ~285 GB/s (measured), so this is the
    dominant term.
  - PSUM: the linear stream needs its 8 banks for the whole k-range,
    but mlp1 needs one transiently.  m-tile 0 of the linear GEMM runs
    from a private contiguous side-stream (saeTd is host-swizzled
    (m, kt, b)-major so m0's columns are one 2 MiB read) AFTER mlp1
    releases its bank; the other 7 m-tiles stream normally.  interT
    comes from SBUF->SBUF DMA transposes (no PE/PSUM needed).
  - Engine/queue plan: sync = sfull + pin partial stores + RS + rsin +
    interT transposes + h_mine + sam0 + out; scalar(ACT) = ew + consts
    + sa stream + htall transposes + final drains; gpsimd = lw stream
    (never behind a collective) + AllGather trigger.  DVE does all FM
    squares, partial drains, inter, and relu so the ACT queue is pure
    DMA until the tail.
"""

import numpy as np
import ml_dtypes

import concourse.bass as bass
import concourse.mybir as mybir
import concourse.tile as tile
from concourse import bacc
from concourse.bass_utils import run_bass_kernel_spmd

B, N, D, O = 1024, 16384, 512, 4096
C = 8                # cores
BS = B // C          # 128 batch rows per core (batch shard after RS)
OS = O // C          # 512 output cols per core
BF16 = mybir.dt.bfloat16
FP8 = mybir.dt.float8e4
F32 = mybir.dt.float32

KT = N // 128        # 128 n-ktiles total
KTL = KT // C        # 16 local ktiles per core (FM contraction shard)
DT = D // 128        # 4 d-tiles
MT = B // 128        # 8 m-tiles (batch) for p2
NB = 16              # n-blocks for the linear GEMM stream
NTB = KT // NB       # 8 n-tiles per block
SFP = 8              # sfull load split
D2 = 2 * D

LIN_SCALE = 4096.0   # 8 (sae shift scale) * 512 (w scale)


def _build(repeat=1, phases=("fm", "ag", "p2", "tail")):
    nc = bacc.Bacc(
        "TRN2",
        target_bir_lowering=False,
        debug=False,
        enable_asserts=False,
        num_devices=C,
    )

    # linear-GEMM sae, fp8, split into the m1..7 stream (kt-major, so a
    # k-block read is one 7 KiB contiguous run per partition) and m0's
    # private side-stream (fully contiguous per partition)
    saeTs = nc.dram_tensor("saeTs", [128, KT * (MT - 1) * 128], FP8,
                           kind="ExternalInput").ap()
    sam0d = nc.dram_tensor("sam0d", [128, KT * 128], FP8,
                           kind="ExternalInput").ap()
    # per-core FM shards (host pre-sliced): 16 ktiles, kt-major
    saebf = nc.dram_tensor("saebf", [128, KTL * B], BF16,
                           kind="ExternalInput").ap()
    embd = nc.dram_tensor("embd", [128, KTL * D], BF16,
                          kind="ExternalInput").ap()
    linwd = nc.dram_tensor("linwd", [128, KT * OS], FP8,
                           kind="ExternalInput").ap()
    mlp1wT = nc.dram_tensor("mlp1wT", [D, D], BF16, kind="ExternalInput").ap()
    mlp1brow = nc.dram_tensor("mlp1brow", [1, D], BF16,
                              kind="ExternalInput").ap()
    mlp2wT = nc.dram_tensor("mlp2wT", [D, OS], BF16, kind="ExternalInput").ap()
    biasrow = nc.dram_tensor("biasrow", [1, OS], BF16,
                             kind="ExternalInput").ap()
    out = nc.dram_tensor("out", [B, OS], BF16, kind="ExternalOutput").ap()

    M7 = (MT - 1) * 128  # 896 stream columns per ktile

    with tile.TileContext(nc) as tc:
      for rep in range(repeat):
        with (
            tc.tile_pool(name=f"dram{rep}", bufs=1, space="DRAM") as dram,
            tc.tile_pool(name=f"const{rep}", bufs=1) as cst,
            tc.tile_pool(name=f"p2sa{rep}", bufs=3) as p2sa,
            tc.tile_pool(name=f"p2lw{rep}", bufs=7) as p2lw,
            tc.tile_pool(name=f"p2st{rep}", bufs=4) as p2st,
        ):
            # ---- first 7 lw blocks emitted BEFORE the FM section so they
            # sit ahead of the ReduceScatter in the gpsimd instruction
            # stream (the RS blocks gpsimd ~63..80us waiting on the FM
            # partials; blocks 7.. are emitted after it and issue at ~80,
            # still ahead of when the PE needs them)
            LW_PRE = 7
            lw_tiles = []

            def emit_lw(nb):
                lw = p2lw.tile([128, NTB, OS], FP8, tag="lw", name="lw")
                nc.gpsimd.dma_start(
                    lw[:], linwd[:, nb * NTB * OS:(nb + 1) * NTB * OS])
                lw_tiles.append(lw)

            if "p2" in phases:
                for nb in range(LW_PRE):
                    emit_lw(nb)

            # const tiles (scalar queue, behind the first ew chunks)
            w1 = cst.tile([128, DT * D], BF16, tag="w1", name="w1")
            b1r = cst.tile([1, D], BF16, tag="b1r", name="b1r")
            w2 = cst.tile([128, DT * OS], BF16, tag="w2", name="w2")
            br = cst.tile([1, OS], BF16, tag="br", name="br")
            ones = cst.tile([1, 128], BF16, tag="ones", name="ones")
            nc.vector.memset(ones[:], 1.0)

            # h gathered from all cores: [B, D] bf16 (AllGather output)
            h_all = dram.tile([C * BS, D], BF16, tag="h_all",
                              name=f"h_all{rep}", addr_space="Shared")
            h_mine = dram.tile([BS, D], BF16, tag="h_mine",
                               name=f"h_mine{rep}")
            htall = cst.tile([128, DT * B], BF16, tag="htall", name="htall")

            # FM partial-sum exchange buffers
            pin = dram.tile([B, D2], BF16, tag="pin", name=f"pin{rep}")
            rsd = dram.tile([BS, D2], BF16, tag="rsd", name=f"rsd{rep}")

            rsin = cst.tile([128, D2], BF16, tag="rsin", name="rsin")
            inter = cst.tile([128, D], BF16, tag="inter", name="inter")
            interT = cst.tile([128, D], BF16, tag="interT", name="interT")
            hbf = cst.tile([128, D], BF16, tag="hbf", name="hbf")

            # ---------------- Phase FM: N-sharded FM GEMMs ----------------
            if "fm" in phases:
              with (
                tc.tile_pool(name=f"fmw{rep}", bufs=1) as fmw,
                tc.tile_pool(name=f"fmps{rep}", bufs=2, space="PSUM") as fmps,
                tc.tile_pool(name=f"fmst{rep}", bufs=3) as fmst,
              ):
                # ew first on scalar so FM matmuls can start early
                ew = fmw.tile([128, KTL, D], BF16, tag="ew", name="ew")
                esq = fmw.tile([128, KTL, D], BF16, tag="esq", name="esq")
                for chv in range(4):
                    ksl = slice(chv * 4, (chv + 1) * 4)
                    nc.scalar.dma_start(ew[:, ksl, :],
                                        embd[:, chv * 4 * D:(chv + 1) * 4 * D])
                    nc.vector.tensor_mul(esq[:, ksl, :], ew[:, ksl, :],
                                         ew[:, ksl, :])

                sfull = fmw.tile([128, KTL, B], BF16, tag="sfull",
                                 name="sfull")
                ssq = fmw.tile([128, KTL, B], BF16, tag="ssq", name="ssq")
                kpp = KTL // SFP
                for s in range(SFP):
                    ksl = slice(s * kpp, (s + 1) * kpp)
                    nc.sync.dma_start(
                        sfull[:, ksl, :],
                        saebf[:, s * kpp * B:(s + 1) * kpp * B])
                    nc.vector.tensor_mul(ssq[:, ksl, :], sfull[:, ksl, :],
                                         sfull[:, ksl, :])

                nc.scalar.dma_start(
                    w1[:], mlp1wT.rearrange("(k p) d -> p k d", p=128))
                nc.scalar.dma_start(b1r[:], mlp1brow[:, :])
                nc.scalar.dma_start(
                    w2[:], mlp2wT.rearrange("(k p) o -> p k o", p=128))
                nc.scalar.dma_start(br[:], biasrow[:, :])

                for m in range(MT):
                    msl = slice(m * 128, (m + 1) * 128)
                    pse = fmps.tile([128, D], F32, tag="pse", name="pse")
                    psq = fmps.tile([128, D], F32, tag="psq", name="psq")
                    for k in range(KTL):
                        nc.tensor.matmul(
                            pse[:], sfull[:, k, msl], ew[:, k, :],
                            start=(k == 0), stop=(k == KTL - 1),
                            skip_group_check=True,
                        )
                        nc.tensor.matmul(
                            psq[:], ssq[:, k, msl], esq[:, k, :],
                            start=(k == 0), stop=(k == KTL - 1),
                            skip_group_check=True,
                        )
                    part = fmst.tile([128, D2], BF16, tag="part", name="part")
                    nc.vector.tensor_copy(part[:, :D], pse[:])
                    nc.vector.tensor_copy(part[:, D:], psq[:])
                    nc.sync.dma_start(pin[m * 128:(m + 1) * 128, :], part[:])

              # sum partials across cores; core c receives rows c*128..
              nc.gpsimd.collective_compute(
                  "ReduceScatter",
                  mybir.AluOpType.add,
                  replica_groups=[list(range(C))],
                  ins=[pin.opt()],
                  outs=[rsd.opt()],
              )
              nc.sync.dma_start(rsin[:], rsd[:, :])

              # inter = sum_emb^2 - sum_sq (0.5 folded into w1), on DVE
              itmp = cst.tile([128, D], F32, tag="itmp", name="itmp")
              nc.vector.tensor_mul(itmp[:], rsin[:, :D], rsin[:, :D])
              nc.vector.tensor_sub(inter[:], itmp[:], rsin[:, D:])
              # interT via SBUF->SBUF DMA transpose (no PE/PSUM)
              for dc in range(DT):
                  nc.sync.dma_start_transpose(
                      interT[:, dc * 128:(dc + 1) * 128],
                      inter[:, dc * 128:(dc + 1) * 128])
            else:
                nc.scalar.dma_start(
                    w1[:], mlp1wT.rearrange("(k p) d -> p k d", p=128))
                nc.scalar.dma_start(b1r[:], mlp1brow[:, :])
                nc.scalar.dma_start(
                    w2[:], mlp2wT.rearrange("(k p) o -> p k o", p=128))
                nc.scalar.dma_start(br[:], biasrow[:, :])
                nc.vector.memset(interT[:], 0.01)

            # ---------------- p2 PSUM pools + mlp1 interleave -------------
            # stack: psm7..psm1 (7 banks), hps (1 bank, transient for mlp1),
            # then psm0 takes hps' bank after mlp1 completes.
            p2ps_cms = [None] * MT
            p2ps_pools = [None] * MT
            for m in range(MT - 1, 0, -1):
                p2ps_cms[m] = tc.tile_pool(
                    name=f"p2ps{rep}_{m}", bufs=1, space="PSUM")
                p2ps_pools[m] = p2ps_cms[m].__enter__()
            psm = [None] * MT
            for m in range(1, MT):
                psm[m] = p2ps_pools[m].tile([128, OS], F32, tag=f"psm{m}",
                                            name=f"psm{m}")

            # mlp1: h = relu(interT.T @ w1 + b1) for OUR batch shard
            hps_cm = tc.tile_pool(name=f"hps{rep}", bufs=1, space="PSUM")
            hps_pool = hps_cm.__enter__()
            hps = hps_pool.tile([128, D], F32, tag="hps", name="hps")
            for kd in range(DT):
                nc.tensor.matmul(
                    hps[:], interT[:, kd * 128:(kd + 1) * 128],
                    w1[:, kd * D:(kd + 1) * D],
                    start=(kd == 0), stop=False, skip_group_check=True,
                )
            nc.tensor.matmul(
                hps[:], ones[:, :], b1r[:, :],
                start=False, stop=True, skip_group_check=True,
            )
            # relu on DVE (ACT queue stays pure-DMA until the tail)
            nc.vector.tensor_scalar_max(hbf[:], hps[:], 0.0)
            nc.sync.dma_start(h_mine[:, :], hbf[:])
            hps_cm.__exit__(None, None, None)

            p2ps_cms[0] = tc.tile_pool(name=f"p2ps{rep}_0", bufs=1,
                                       space="PSUM")
            p2ps_pools[0] = p2ps_cms[0].__enter__()
            psm[0] = p2ps_pools[0].tile([128, OS], F32, tag="psm0",
                                        name="psm0")

            # ---------------- Phase p2: fp8 DoubleRow linear GEMM ---------
            if "p2" in phases:
                # m-tiles 1..7 ride the shared sa stream
                for nb in range(NB):
                    if nb + LW_PRE < NB:
                        emit_lw(nb + LW_PRE)
                    sa = p2sa.tile([128, NTB, MT - 1, 128], FP8, tag="sa",
                                   name="sa")
                    nc.scalar.dma_start(
                        sa[:], saeTs[:, nb * NTB * M7:(nb + 1) * NTB * M7])
                    lw = lw_tiles[nb]
                    for mi in range(MT - 1):
                        for jp in range(NTB // 2):
                            nc.tensor.matmul(
                                psm[mi + 1][:],
                                sa[:, 2 * jp:2 * jp + 2, mi, :],
                                lw[:, 2 * jp:2 * jp + 2, :],
                                start=(nb == 0 and jp == 0),
                                stop=False,
                                perf_mode=mybir.MatmulPerfMode.DoubleRow,
                                skip_group_check=True,
                            )

                # m0 from its private contiguous side-stream (after mlp1
                # released its PSUM bank)
                sam0 = cst.tile([128, KT, 128], FP8, tag="sam0", name="sam0")
                for hh in range(2):
                    ksl = slice(hh * (KT // 2) * 128, (hh + 1) * (KT // 2) * 128)
                    nc.sync.dma_start(
                        sam0[:, hh * (KT // 2):(hh + 1) * (KT // 2), :],
                        sam0d[:, ksl])
                for nb in range(NB):
                    for jp in range(NTB // 2):
                        j0 = nb * NTB + 2 * jp
                        nc.tensor.matmul(
                            psm[0][:],
                            sam0[:, j0:j0 + 2, :],
                            lw_tiles[nb][:, 2 * jp:2 * jp + 2, :],
                            start=(nb == 0 and jp == 0),
                            stop=False,
                            perf_mode=mybir.MatmulPerfMode.DoubleRow,
                            skip_group_check=True,
                        )
            else:
                for m in range(MT):
                    nc.tensor.matmul(
                        psm[m][:], ones[:, :], br[:, :],
                        start=True, stop=False, skip_group_check=True,
                    )

            # AllGather of h (emitted after the lw stream: its wait on h
            # cannot head-block anything that matters on gpsimd now)
            if "ag" in phases and "fm" in phases:
                nc.gpsimd.collective_compute(
                    "AllGather",
                    mybir.AluOpType.bypass,
                    replica_groups=[list(range(C))],
                    ins=[h_mine.opt()],
                    outs=[h_all.opt()],
                )
                for kd in range(DT):
                    nc.scalar.dma_start_transpose(
                        htall[:, kd * B:(kd + 1) * B],
                        h_all[:, kd * 128:(kd + 1) * 128])
            else:
                nc.vector.memset(htall[:], 0.01)

            # ---------------- tail: mlp2 + bias + drain -------------------
            for m in range(MT):
                if "tail" in phases:
                    for kd in range(DT):
                        nc.tensor.matmul(
                            psm[m][:],
                            htall[:, kd * B + m * 128:kd * B + (m + 1) * 128],
                            w2[:, kd * OS:(kd + 1) * OS],
                            start=False, stop=False,
                            skip_group_check=True,
                        )
                nc.tensor.matmul(
                    psm[m][:], ones[:, :], br[:, :],
                    start=False, stop=True, skip_group_check=True,
                )
                ot = p2st.tile([128, OS], BF16, tag="ot", name="ot")
                nc.scalar.activation(
                    ot[:], psm[m][:],
                    mybir.ActivationFunctionType.Copy,
                    scale=1.0 / LIN_SCALE)
                nc.sync.dma_start(out[m * 128:(m + 1) * 128, :], ot[:])
                p2ps_cms[m].__exit__(None, None, None)

    nc.compile()
    return nc


_CACHE = {}


def _get_nc():
    if "nc" not in _CACHE:
        _CACHE["nc"] = _build()
    return _CACHE["nc"]


def _swz(a, inner):
    """[K*128, inner] row-major -> [128, K*inner] partition-major flat."""
    k = a.shape[0] // 128
    return np.ascontiguousarray(
        a.reshape(k, 128, inner).transpose(1, 0, 2).reshape(128, k * inner))


def make_in_maps(sae_features, emb, linear_w, linear_b, mlp1_w, mlp1_b,
                 mlp2_w, mlp2_b):
    bf = ml_dtypes.bfloat16
    e4 = ml_dtypes.float8_e4m3
    f32 = np.float32
    sae = np.asarray(sae_features, dtype=f32)
    emb_f = np.asarray(emb, dtype=f32)
    W = np.asarray(linear_w, f32)

    saeT = np.ascontiguousarray(sae.T)              # (N, B) f32
    saeT8 = ((saeT - 0.5) * 8.0).astype(e4)         # (N, B) fp8, mean-shifted
    # m1..7 stream, kt-major: [128, KT*896]; m0 side-stream: [128, KT*128]
    saeTs = _swz(np.ascontiguousarray(saeT8[:, 128:]), (MT - 1) * 128)
    sam0d = _swz(np.ascontiguousarray(saeT8[:, :128]), 128)
    saeT_bf = saeT.astype(bf)                        # (N, B) bf16
    embd_full = _swz(emb_f.astype(bf), D)            # [128, KT*D]
    saebf_full = _swz(saeT_bf, B)                    # [128, KT*B]
    mlp1wT = np.ascontiguousarray((0.5 * np.asarray(mlp1_w, f32)).T).astype(bf)
    mlp1brow = np.asarray(mlp1_b, f32).reshape(1, D).astype(bf)
    mlp2wT_f = np.ascontiguousarray(np.asarray(mlp2_w, f32).T) * LIN_SCALE
    linwT_f = np.ascontiguousarray(W.T) * 512.0     # (N, O) f32, fp8-scaled
    # fused bias: linear_b + mlp2_b + the mean-shift correction, PSUM scale
    bias_f = (np.asarray(linear_b, f32) + np.asarray(mlp2_b, f32)
              + 0.5 * W.sum(axis=1)) * LIN_SCALE    # (O,)

    in_maps = []
    for c in range(C):
        osl = slice(c * OS, (c + 1) * OS)
        in_maps.append({
            "saeTs": saeTs,
            "sam0d": sam0d,
            "saebf": np.ascontiguousarray(
                saebf_full[:, c * KTL * B:(c + 1) * KTL * B]),
            "embd": np.ascontiguousarray(
                embd_full[:, c * KTL * D:(c + 1) * KTL * D]),
            "linwd": _swz(
                np.ascontiguousarray(linwT_f[:, osl]).astype(e4), OS),
            "mlp1wT": mlp1wT,
            "mlp1brow": mlp1brow,
            "mlp2wT": np.ascontiguousarray(mlp2wT_f[:, osl]).astype(bf),
            "biasrow": bias_f[osl].reshape(1, OS).astype(bf),
        })
    return in_maps


def kernel(sae_features, emb, linear_w, linear_b, mlp1_w, mlp1_b, mlp2_w,
           mlp2_b):
    nc = _get_nc()
    in_maps = make_in_maps(
        sae_features, emb, linear_w, linear_b, mlp1_w, mlp1_b, mlp2_w, mlp2_b
    )
    res = run_bass_kernel_spmd(nc, in_maps, list(range(C)))
    full = np.empty((B, O), dtype=np.float32)
    for c in range(C):
        full[:, c * OS:(c + 1) * OS] = res.results[c]["out"].astype(np.float32)
    return full
